# revision 6
# baseline (speedup 1.0000x reference)
"""Adaptive linear (per-batch expert weight gather + matmul + bias) on 8 TRN2 cores.

Reference semantics:
    out[b, n, o] = sum_k x[b, n, k] * weight[indices[b], k, o] + bias[indices[b], 0, o]
with x [256, 1024, 256], indices [256], weight [1024, 256, 256], bias [1024, 1, 256].

Sharding: data-parallel over the batch dim B=256 -> 32 batches per core. The
weight/bias tables are replicated to every core; each core gathers the 32
weight tiles it needs ON DEVICE via indirect DMA driven by its indices, then
runs float32r matmuls (w stationary, x moving) accumulating in PSUM, adds the
gathered bias during the PSUM drain, and writes out.

Layout choices (host-side, pure layout/sharding transforms):
  - x is passed per-core transposed to [IN, BL*N] so the contraction dim (IN)
    lands on SBUF partitions without any on-device transpose.
  - output is produced as out^T per batch ([OUT, BL*N]) and transposed back on
    the host after gathering.
  - the weight table is passed as rows [C*IN, OUT]; the gather offset vectors
    (idx*IN + k) are precomputed on the host from the indices (pure index
    arithmetic; the data movement they drive happens on device).
"""

import numpy as np

from concourse import bacc, bass, mybir, tile
from concourse.bass_utils import run_bass_kernel_spmd
from concourse.masks import make_identity

NCORES = 8
B, N, IN, OUT, C = 256, 1024, 256, 256, 1024
BL = B // NCORES          # 32 batches per core
KC = IN // 128            # 2 contraction chunks
MC = OUT // 128           # 2 output-partition chunks
FD = 512                  # fp32 matmul moving-operand max / one PSUM bank
FC = N // FD              # 2 free chunks
NG = BL * KC              # 64 weight-gather calls per core
NB = 2                    # batches per x/out DMA group

_F32 = mybir.dt.float32
_F32R = mybir.dt.float32r
_I32 = mybir.dt.int32

_nc_cache = []
_last_in_maps = None


def _build():
    nc = bacc.Bacc("TRN2", target_bir_lowering=False, debug=False, num_devices=NCORES)
    x_t = nc.dram_tensor("x_t", [IN, BL * N], _F32, kind="ExternalInput").ap()
    wtab = nc.dram_tensor("wtab", [C * IN, OUT], _F32, kind="ExternalInput").ap()
    btab = nc.dram_tensor("btab", [C, OUT], _F32, kind="ExternalInput").ap()
    woff = nc.dram_tensor("woff", [128, NG], _I32, kind="ExternalInput").ap()
    idx = nc.dram_tensor("idx", [BL], _I32, kind="ExternalInput").ap()
    out_t = nc.dram_tensor("out_t", [OUT, BL * N], _F32, kind="ExternalOutput").ap()

    with tile.TileContext(nc) as tc:
        with (
            tc.tile_pool(name="sb", bufs=1) as sb,
            tc.tile_pool(name="wp", bufs=1) as wp,
            tc.tile_pool(name="xp", bufs=1) as xp,
            tc.tile_pool(name="op", bufs=1) as op,
            tc.tile_pool(name="psp", bufs=1, space="PSUM") as psp,
        ):
            offs = sb.tile([128, NG], _I32, tag="offs", bufs=1)
            nc.sync.dma_start(offs[:], woff[:])
            idxt = sb.tile([BL, 1], _I32, tag="idxt", bufs=1)
            nc.sync.dma_start(idxt[:], idx[0:BL, None])

            # bias: gather the 32 rows, then PE-transpose to [OUT-chunk, BL]
            ident = sb.tile([128, 128], _F32, tag="ident", bufs=1)
            make_identity(nc, ident[:])
            bsb = sb.tile([BL, OUT], _F32, tag="bsb", bufs=1)
            nc.gpsimd.indirect_dma_start(
                out=bsb[:],
                out_offset=None,
                in_=btab[:, :],
                in_offset=bass.IndirectOffsetOnAxis(ap=idxt[:, :1], axis=0),
            )
            bt = []
            for mc in range(MC):
                pst = psp.tile([128, BL], _F32, tag="tr", bufs=2)
                nc.tensor.transpose(
                    out=pst[:],
                    in_=bsb[:BL, mc * 128 : (mc + 1) * 128],
                    identity=ident[:BL, :BL],
                )
                btile = sb.tile([128, BL], _F32, tag="bt", bufs=2)
                nc.vector.tensor_copy(btile[:], pst[:])
                bt.append(btile)

            # gather all per-batch weight tiles up front (independent of x)
            wt = []
            for b in range(BL):
                per = []
                for kc in range(KC):
                    r = b * KC + kc
                    w = wp.tile([128, OUT], _F32, tag="w", bufs=NG)
                    nc.gpsimd.indirect_dma_start(
                        out=w[:],
                        out_offset=None,
                        in_=wtab[:, :],
                        in_offset=bass.IndirectOffsetOnAxis(ap=offs[:, r : r + 1], axis=0),
                    )
                    per.append(w)
                wt.append(per)

            for bg in range(0, BL, NB):
                xs = []
                for kc in range(KC):
                    xt_ = xp.tile([128, NB * N], _F32, tag=f"x{kc}", bufs=2)
                    nc.sync.dma_start(
                        xt_[:], x_t[kc * 128 : (kc + 1) * 128, bg * N : (bg + NB) * N]
                    )
                    xs.append(xt_)
                os_ = []
                for mc in range(MC):
                    ot = op.tile([128, NB * N], _F32, tag=f"o{mc}", bufs=2)
                    os_.append(ot)
                for j in range(NB):
                    b = bg + j
                    for mc in range(MC):
                        pss = []
                        for f in range(FC):
                            ps_mm = psp.tile(
                                [128, FD], _F32, tag="mm", bufs=6, name=f"mm_{b}_{mc}_{f}"
                            )
                            pss.append(ps_mm)
                        for kc in range(KC):
                            lhsT = wt[b][kc][:, mc * 128 : (mc + 1) * 128]
                            for f in range(FC):
                                rhs = xs[kc][
                                    :, j * N + f * FD : j * N + (f + 1) * FD
                                ]
                                nc.tensor.matmul(
                                    pss[f][:],
                                    lhsT,
                                    rhs,
                                    start=(kc == 0),
                                    stop=(kc == KC - 1),
                                )
                        for f in range(FC):
                            nc.vector.tensor_tensor(
                                out=os_[mc][:, j * N + f * FD : j * N + (f + 1) * FD],
                                in0=pss[f][:],
                                in1=bt[mc][:, b : b + 1].to_broadcast([128, FD]),
                                op=mybir.AluOpType.add,
                            )
                for mc in range(MC):
                    nc.sync.dma_start(
                        out_t[mc * 128 : (mc + 1) * 128, bg * N : (bg + NB) * N],
                        os_[mc][:],
                    )

    nc.compile()
    return nc


def _get_nc():
    if not _nc_cache:
        _nc_cache.append(_build())
    return _nc_cache[0]


def kernel(x, indices, weight, bias):
    x = np.asarray(x, dtype=np.float32)
    idx_np = np.asarray(indices).astype(np.int64).reshape(B)
    wtab = np.ascontiguousarray(np.asarray(weight, dtype=np.float32)).reshape(
        C * IN, OUT
    )
    btab = np.ascontiguousarray(np.asarray(bias, dtype=np.float32)).reshape(C, OUT)

    nc = _get_nc()

    in_maps = []
    for c in range(NCORES):
        sl = slice(c * BL, (c + 1) * BL)
        xs = np.ascontiguousarray(np.transpose(x[sl], (2, 0, 1))).reshape(IN, BL * N)
        il = idx_np[sl].astype(np.int32)
        woff = (
            il[None, :, None] * IN
            + np.arange(KC, dtype=np.int32)[None, None, :] * 128
            + np.arange(128, dtype=np.int32)[:, None, None]
        ).astype(np.int32)
        in_maps.append(
            {
                "x_t": xs,
                "wtab": wtab,
                "btab": btab,
                "woff": woff.reshape(128, NG),
                "idx": il,
            }
        )

    global _last_in_maps
    _last_in_maps = in_maps

    res = run_bass_kernel_spmd(nc, in_maps, core_ids=list(range(NCORES)))

    outs = []
    for c in range(NCORES):
        ot = res.results[c]["out_t"].reshape(OUT, BL, N)
        outs.append(np.transpose(ot, (1, 2, 0)))
    return np.ascontiguousarray(np.concatenate(outs, axis=0))
